# revision 11
# baseline (speedup 1.0000x reference)
"""Trainium2 Bass kernel for nn_AttentionWithVQ (B=4, N=2048, DIM=512, H=8,
depthwise-conv positional term, softmax attention, output projection).

Sharding: data-parallel over B (4 batches x 2 core-groups) and tensor-parallel
over heads (4 heads per core) -> 8 cores, fully independent per core except a
final partial-sum over the two head-groups of each batch, done on host at
gather time (the output projection contracts over heads).

Core algorithmic fusion: the score matrix
    S = 0.5*(scale * q @ k^T + scale * conv1(m) @ conv2(s)^T)
is ONE matmul over a concatenated 128-feature axis:
    S = Qp^T @ Kp,  Qp = [q*scale*0.5 ; conv1(m)*scale*0.5], Kp = [k ; conv2(s)]
which exactly fills the 128x128 PE array contraction dim.

Softmax denominators come for free by appending a ones-column to V
(attn@[V|1] yields the row-sums of exp(S) in the last output row); exp() is
numerically safe without max-subtraction for this problem's score magnitudes.

Schedule: the kernel is paced by the Scalar engine's 128 exp() instructions
(the hard floor at ~1.1us each).  Everything else is arranged around keeping
that stream dense:
  - minimal prologue: only the qkv chunks needed by head 0/1 stripe 0 run
    before the first exp; v-projection, the remaining qkv chunks, the t=1
    convs and the previous stripe's output projection are emitted as PE/DVE
    "fillers" inside the attention nk-loops.
  - loop order stripe-outer/head-inner so each stripe's projection + output
    DMA overlaps the next stripe's attention (no serialized tail).
  - per-(head,stripe) softmax normalization (reciprocal + DRAM-bounce
    partition broadcast) overlapped with the next head's attention.

Partition alignment: compute engines are lane-locked, so per-head feature
layouts alternate by head parity (even heads [qk;conv], odd heads [conv;qk])
making every PSUM->SBUF copy partition-aligned; the few genuinely shifting
copies (odd-head attention outputs, denominator broadcast) go through DMA.
"""

import os
import sys

sys.path.insert(0, "/opt/trn_rl_repo")

import numpy as np

# ---------------------------------------------------------------- constants
B, N, DIM, HEAD, VQE_K = 4, 2048, 512, 8, 3
Dh = DIM // HEAD            # 64
HPC = HEAD // 2             # heads per core (8 cores = 4 batch * 2 groups)
P = 128
NKB = N // P                # 16 key blocks
FB = 512                    # one fp32 PSUM bank
FBS = 1024                  # attention stripe chunk (2 banks)
NST = N // FBS              # 2 q-stripes
SCALE_Q = Dh ** -0.5 * 0.5  # folds the 0.5 score scale into the q/conv1 side

_DEFAULT_CFG = {}
_CACHE = {}


# ---------------------------------------------------------------- host prep
def _host_prep(core, inp):
    """Build the per-core input arrays (sharding + layout permutations)."""
    import ml_dtypes

    bf16 = ml_dtypes.bfloat16
    b, g = core // 2, core % 2
    f32 = np.float32
    x, m, s = inp["x"], inp["m"], inp["s"]
    qkv_w, qkv_b = inp["qkv_w"], inp["qkv_b"]
    proj_w = inp["proj_w"]
    p1w = inp["pe1_w"].reshape(HEAD, VQE_K)
    p2w = inp["pe2_w"].reshape(HEAD, VQE_K)
    pe1_b, pe2_b = inp["pe1_b"], inp["pe2_b"]

    d = {}
    d["xt"] = np.ascontiguousarray(x[b].T).astype(bf16)  # [512, 2048]

    # m/s transposed, tile t rows = [head(2t+1) feats ; head(2t) feats]
    mt = np.empty((256, N), f32)
    st = np.empty((256, N), f32)
    mcw = np.zeros((128, 8), f32)
    scw = np.zeros((128, 8), f32)
    for t in range(2):
        h_lo, h_hi = g * 4 + 2 * t + 1, g * 4 + 2 * t
        mt[t * 128:t * 128 + 64] = m[b][:, h_lo * 64:(h_lo + 1) * 64].T
        mt[t * 128 + 64:t * 128 + 128] = m[b][:, h_hi * 64:(h_hi + 1) * 64].T
        st[t * 128:t * 128 + 64] = s[b][:, h_lo * 64:(h_lo + 1) * 64].T
        st[t * 128 + 64:t * 128 + 128] = s[b][:, h_hi * 64:(h_hi + 1) * 64].T
        for p in range(128):
            h = g * 4 + 2 * t + (1 if p < 64 else 0)
            mcw[p, 4 * t:4 * t + 3] = p1w[h] * SCALE_Q
            scw[p, 4 * t:4 * t + 3] = p2w[h]
            mcw[p, 4 * t + 3] = pe1_b[h] * SCALE_Q
            scw[p, 4 * t + 3] = pe2_b[h]
    d["mt"], d["st"] = mt.astype(bf16), st.astype(bf16)
    d["mcw"], d["scw"] = mcw, scw

    # q/k projection weights: chunk ch=(t, q|k) = [even-head rows; odd-head rows]
    wqk_f = np.empty((512, DIM), f32)
    qkb = np.zeros((128, 4), f32)
    for t in range(2):
        for j in range(2):  # 0=q, 1=k
            ch = 2 * t + j
            h_e, h_o = g * 4 + 2 * t, g * 4 + 2 * t + 1
            base = j * DIM
            wqk_f[ch * 128:ch * 128 + 64] = qkv_w[base + h_e * 64:base + (h_e + 1) * 64]
            wqk_f[ch * 128 + 64:(ch + 1) * 128] = qkv_w[base + h_o * 64:base + (h_o + 1) * 64]
            qkb[0:64, ch] = qkv_b[base + h_e * 64:base + (h_e + 1) * 64]
            qkb[64:128, ch] = qkv_b[base + h_o * 64:base + (h_o + 1) * 64]
            if j == 0:
                wqk_f[ch * 128:(ch + 1) * 128] *= SCALE_Q
                qkb[:, ch] *= SCALE_Q
    d["wqk"] = np.ascontiguousarray(wqk_f.T).astype(bf16)  # [c=512, f=512]
    d["qkb"] = qkb

    d["wv"] = np.ascontiguousarray(
        qkv_w[2 * DIM + g * 256:2 * DIM + (g + 1) * 256].T).astype(bf16)  # [512, 256]
    # v bias replicated along partitions: column order matches wv columns
    vb = qkv_b[2 * DIM + g * 256:2 * DIM + (g + 1) * 256]
    d["vbrep"] = np.broadcast_to(vb, (128, 256)).astype(bf16).copy()

    # proj rows in aT partition order: aT tile t partition p -> head
    # 2t+(p>=64), d=p%64
    pjt = np.empty((256, DIM), f32)
    for t in range(2):
        for p in range(128):
            h_l = 2 * t + (1 if p >= 64 else 0)
            h = g * 4 + h_l
            pjt[t * 128 + p] = proj_w[:, h * 64 + (p % 64)]
    d["pjt"] = pjt.astype(bf16)
    return d


# ------------------------------------------------------------- device build
def _emit(tc, nc, io):
    from contextlib import ExitStack

    from concourse import mybir

    dt = mybir.dt
    f32 = dt.float32
    bf16 = dt.bfloat16
    AF = mybir.ActivationFunctionType
    ALU = mybir.AluOpType

    with ExitStack() as ctx:
        persist = ctx.enter_context(tc.tile_pool(name="persist", bufs=1))
        xtp = ctx.enter_context(tc.tile_pool(name="xtp", bufs=1))
        convp = ctx.enter_context(tc.tile_pool(name="convp", bufs=2))
        convyp = ctx.enter_context(tc.tile_pool(name="convyp", bufs=2))
        # PSUM: s_pool 2x2 banks, o_pool 1x2 banks, shp 2x1 bank = 8 banks
        s_pool = ctx.enter_context(
            tc.tile_pool(name="s_pool", bufs=2, space="PSUM"))
        o_pool = ctx.enter_context(
            tc.tile_pool(name="o_pool", bufs=1, space="PSUM"))
        shp = ctx.enter_context(tc.tile_pool(name="shp", bufs=2, space="PSUM"))
        esb = ctx.enter_context(tc.tile_pool(name="esb", bufs=8))
        stgp = ctx.enter_context(tc.tile_pool(name="stgp", bufs=2))
        denp = ctx.enter_context(tc.tile_pool(name="denp", bufs=2))
        bcp = ctx.enter_context(tc.tile_pool(name="bcp", bufs=2))
        obp = ctx.enter_context(tc.tile_pool(name="obp", bufs=3))

        # ---- persistent tiles
        wqk_sb = [persist.tile([128, 512], bf16, name=f"wqk{c}", tag=f"wqk{c}")
                  for c in range(4)]
        wv_sb = [persist.tile([128, 256], bf16, name=f"wv{c}", tag=f"wv{c}")
                 for c in range(4)]
        pjt_sb = [persist.tile([128, 512], bf16, name=f"pjt{f}", tag=f"pjt{f}")
                  for f in range(2)]
        mcw_sb = persist.tile([128, 8], f32, name="mcw", tag="mcw")
        scw_sb = persist.tile([128, 8], f32, name="scw", tag="scw")
        qkb_sb = persist.tile([128, 4], f32, name="qkb", tag="qkb")
        vbr_sb = persist.tile([128, 256], bf16, name="vbrep", tag="vbrep")
        QP = [persist.tile([128, N], bf16, name=f"QP{h}", tag=f"QP{h}")
              for h in range(HPC)]
        KP = [persist.tile([128, N], bf16, name=f"KP{h}", tag=f"KP{h}")
              for h in range(HPC)]
        # per-head V block is [v(64) | ones | zero-pad] = 66 columns (even
        # width keeps 4-byte operand alignment for bf16)
        v_sb = [persist.tile([128, HPC * 66], bf16, name=f"vsb{b_}",
                             tag=f"vsb{b_}") for b_ in range(NKB)]
        aT = [persist.tile([128, N], bf16, name=f"aT{t}", tag=f"aT{t}")
              for t in range(2)]
        xt_sb = [xtp.tile([128, N], bf16, name=f"xt{c}", tag=f"xt{c}")
                 for c in range(4)]

        # ---- input DMAs, priority order (sync queue carries the critical
        # prologue loads; weights ride other queues)
        cin0 = {}
        for src in ("st", "mt"):
            cin0[src] = convp.tile([128, N], bf16, name=f"ci_{src}0",
                                   tag="cin")
        nc.sync.dma_start(cin0["st"][:], io["st"][0:128, :])
        nc.sync.dma_start(xt_sb[0][:], io["xt"][0:128, :])
        nc.sync.dma_start(cin0["mt"][:], io["mt"][0:128, :])
        nc.sync.dma_start(xt_sb[1][:], io["xt"][128:256, :])
        nc.gpsimd.dma_start(xt_sb[2][:], io["xt"][256:384, :])
        nc.gpsimd.dma_start(xt_sb[3][:], io["xt"][384:512, :])
        for c in range(4):
            nc.scalar.dma_start(wqk_sb[c][:], io["wqk"][c * 128:(c + 1) * 128, :])
        for c in range(4):
            nc.gpsimd.dma_start(wv_sb[c][:], io["wv"][c * 128:(c + 1) * 128, :])
        nc.scalar.dma_start(pjt_sb[0][:], io["pjt"][0:128, :])
        nc.scalar.dma_start(pjt_sb[1][:], io["pjt"][128:256, :])
        nc.scalar.dma_start(qkb_sb[:], io["qkb"][:, :])
        nc.scalar.dma_start(mcw_sb[:], io["mcw"][:, :])
        nc.scalar.dma_start(scw_sb[:], io["scw"][:, :])
        nc.gpsimd.dma_start(vbr_sb[:], io["vbrep"][:, :])

        # ---- helpers -----------------------------------------------------
        def conv_ops(src, wv_, dst, t, xin=None):
            """Depthwise 3-tap conv along N for tile t of m/s (DVE), writing
            the two parity halves into the QP/KP tiles."""
            if xin is None:
                xin = convp.tile([128, N], bf16, name=f"ci_{src}{t}",
                                 tag="cin")
                nc.sync.dma_start(xin[:], io[src][t * 128:(t + 1) * 128, :])
            y = convyp.tile([128, N], bf16, name=f"cy_{src}{t}", tag="cy")
            w0, w1, w2, cb = (wv_[:, 4 * t + k:4 * t + k + 1] for k in range(4))
            nc.vector.tensor_scalar(y[:], xin[:], w1, cb, ALU.mult, ALU.add)
            nc.vector.scalar_tensor_tensor(
                y[:, 1:], xin[:, :N - 1], w0, y[:, 1:], ALU.mult, ALU.add)
            nc.vector.scalar_tensor_tensor(
                y[:, :N - 1], xin[:, 1:], w2, y[:, :N - 1], ALU.mult, ALU.add)
            nc.vector.tensor_copy(dst[2 * t + 1][0:64, :], y[0:64, :])
            nc.vector.tensor_copy(dst[2 * t][64:128, :], y[64:128, :])

        def qkv_chunk(ch, qs, pool, tag, width):
            """q/k projection chunk ch over q-columns qs (width cols)."""
            t, j = ch // 2, ch % 2
            dst = QP if j == 0 else KP
            ps = pool.tile([128, width], f32, name="psqk", tag=tag)
            nh = width // FB
            for ih in range(nh):
                for c in range(4):
                    nc.tensor.matmul(
                        ps[:, ih * FB:(ih + 1) * FB],
                        wqk_sb[c][:, ch * 128:(ch + 1) * 128],
                        xt_sb[c][:, qs.start + ih * FB:qs.start + (ih + 1) * FB],
                        start=(c == 0), stop=(c == 3))
            nc.vector.tensor_scalar_add(
                dst[2 * t][0:64, qs], ps[0:64, :], qkb_sb[0:64, ch:ch + 1])
            nc.vector.tensor_scalar_add(
                dst[2 * t + 1][64:128, qs], ps[64:128, :],
                qkb_sb[64:128, ch:ch + 1])

        def v_block(blk):
            """v projection for key-block blk + bias + ones/pad columns."""
            bs = slice(blk * 128, (blk + 1) * 128)
            ps = shp.tile([128, 512], f32, name="psv", tag="sh")
            for c in range(4):
                nc.tensor.matmul(ps[:, 0:256], xt_sb[c][:, bs], wv_sb[c][:],
                                 start=(c == 0), stop=(c == 3))
            v3 = v_sb[blk].rearrange("p (h f) -> p h f", h=HPC)
            ps3 = ps.rearrange("p (h f) -> p h f", f=64)
            nc.vector.scalar_tensor_tensor(
                v3[:, :, 0:64], ps3[:, 0:HPC, :],
                1.0, vbr_sb.rearrange("p (h f) -> p h f", h=HPC),
                ALU.mult, ALU.add)
            nc.vector.memset(v3[:, :, 64:65], 1.0)
            nc.vector.memset(v3[:, :, 65:66], 0.0)

        def drain(h, q2, o_ps):
            """Normalize o_ps by its softmax denominators into aT."""
            t, odd = h // 2, h % 2
            cs = slice(q2 * FBS, (q2 + 1) * FBS)
            row = h * NST + q2
            den = denp.tile([128, FBS], bf16, name=f"den{row}", tag="den")
            with nc.allow_low_precision(reason="softmax denom fits bf16"):
                nc.vector.reciprocal(den[64:65, :], o_ps[64:65, :])
            nc.sync.dma_start(io["drec"][row:row + 1, :], den[64:65, :])
            bc = bcp.tile([64, FBS], bf16, name=f"bc{row}", tag="bc")
            nc.sync.dma_start(
                bc[:], io["drec"][row:row + 1, :].broadcast_to([64, FBS]))
            if odd:
                stg = stgp.tile([64, FBS], bf16, name=f"stg{row}", tag="stg")
                nc.vector.tensor_mul(stg[:], o_ps[0:64, :], bc[:])
                nc.sync.dma_start(aT[t][64:128, cs], stg[:])
            else:
                nc.vector.tensor_mul(aT[t][0:64, cs], o_ps[0:64, :], bc[:])

        def proj_block(blk):
            """Output projection for 128 q rows + bf16 store."""
            bs = slice(blk * 128, (blk + 1) * 128)
            pj = shp.tile([128, FB], f32, name="pj", tag="sh")
            nc.tensor.matmul(pj[:], aT[0][:, bs], pjt_sb[0][:],
                             start=True, stop=False)
            nc.tensor.matmul(pj[:], aT[1][:, bs], pjt_sb[1][:],
                             start=False, stop=True)
            ob = obp.tile([128, FB], bf16, name="ob", tag="ob")
            nc.vector.tensor_copy(ob[:], pj[:])
            nc.gpsimd.dma_start(io["out"][bs, :], ob[:])

        # ---- prologue: conv t=0 + the qkv chunks head 0/1 stripe 0 needs
        conv_ops("st", scw_sb, KP, 0, cin0["st"])
        conv_ops("mt", mcw_sb, QP, 0, cin0["mt"])
        qkv_chunk(1, slice(0, 1024), s_pool, "sps", 1024)     # k(t0) cols 0:1024
        qkv_chunk(1, slice(1024, 2048), s_pool, "sps", 1024)  # k(t0) cols 1024:
        qkv_chunk(0, slice(0, 1024), s_pool, "sps", 1024)     # q(t0) cols 0:1024
        v_block(0)
        v_block(1)

        # ---- attention: stripe-outer, head-inner, exp-paced; fillers are
        # (engine, emit-fn) pairs consumed one per nk iteration
        def fillers_for(h, q2):
            fl = []
            if q2 == 0 and h == 0:
                for blk in range(2, NKB):
                    fl.append(lambda b_=blk: v_block(b_))
                # t=1 convs ride the DVE queue during head 0
                fl.insert(6, lambda: conv_ops("st", scw_sb, KP, 1))
                fl.insert(11, lambda: conv_ops("mt", mcw_sb, QP, 1))
            elif q2 == 0 and h == 1:
                for ch in (3, 2):  # k(t1) first, then q(t1)
                    for qb in range(4):
                        fl.append(lambda c_=ch, q_=qb: qkv_chunk(
                            c_, slice(q_ * 512, (q_ + 1) * 512), shp, "sh",
                            512))
            elif q2 == 0 and h == 2:
                # deferred q(t0) cols 1024:2048 (needed by stripe 1)
                fl.append(lambda: qkv_chunk(0, slice(1024, 1536), shp, "sh",
                                            512))
                fl.append(lambda: qkv_chunk(0, slice(1536, 2048), shp, "sh",
                                            512))
            elif q2 == 1 and h < 2:
                # previous stripe's projection (8 blocks over 2 heads)
                for blk in range(h * 4, h * 4 + 4):
                    fl.append(lambda b_=blk: proj_block(b_))
            return fl

        for q2 in range(NST):
            for h in range(HPC):
                vcols = slice(h * 66, (h + 1) * 66)
                cs0 = q2 * FBS
                fl = fillers_for(h, q2)
                o_ps = o_pool.tile([66, FBS], f32, name=f"o{h}_{q2}",
                                   tag="ops")
                for nk in range(NKB):
                    if fl:
                        fl.pop(0)()
                    ks = slice(nk * 128, (nk + 1) * 128)
                    s_ps = s_pool.tile([128, FBS], f32, name="sps", tag="sps")
                    for ih in range(2):
                        nc.tensor.matmul(
                            s_ps[:, ih * FB:(ih + 1) * FB], KP[h][:, ks],
                            QP[h][:, cs0 + ih * FB:cs0 + (ih + 1) * FB],
                            start=True, stop=True)
                    e = esb.tile([128, FBS], bf16, name="e", tag="e")
                    nc.scalar.activation(e[:], s_ps[:], AF.Exp)
                    for ih in range(2):
                        nc.tensor.matmul(
                            o_ps[:, ih * FB:(ih + 1) * FB], v_sb[nk][:, vcols],
                            e[:, ih * FB:(ih + 1) * FB],
                            start=(nk == 0), stop=(nk == NKB - 1))
                while fl:
                    fl.pop(0)()
                drain(h, q2, o_ps)
            if q2 == NST - 1:  # last stripe: projection is the tail
                for blk in range(q2 * 8, q2 * 8 + 8):
                    proj_block(blk)


def _build(cfg_key):
    from concourse import bacc, mybir, tile

    dt = mybir.dt
    nc = bacc.Bacc("TRN2", target_bir_lowering=False, debug=False,
                   num_devices=8)
    shapes = {
        "xt": ([DIM, N], dt.bfloat16),
        "mt": ([256, N], dt.bfloat16), "st": ([256, N], dt.bfloat16),
        "wqk": ([DIM, 512], dt.bfloat16), "wv": ([DIM, 256], dt.bfloat16),
        "pjt": ([256, DIM], dt.bfloat16),
        "mcw": ([128, 8], dt.float32), "scw": ([128, 8], dt.float32),
        "qkb": ([128, 4], dt.float32), "vbrep": ([128, 256], dt.bfloat16),
    }
    io = {}
    for name, (shape, dtt) in shapes.items():
        io[name] = nc.dram_tensor(name, shape, dtt,
                                  kind="ExternalInput").ap()
    io["out"] = nc.dram_tensor("out", [N, DIM], dt.bfloat16,
                               kind="ExternalOutput").ap()
    # internal DRAM bounce for the denominator broadcast (DMA cannot
    # replicate from an SBUF source, but a DRAM source AP is linear and
    # supports a zero-step leading dim)
    io["drec"] = nc.dram_tensor("drec", [HPC * NST, FBS], dt.bfloat16).ap()
    with tile.TileContext(nc) as tc:
        _emit(tc, nc, io)
    nc.compile()
    return nc


def _get_program(cfg=None):
    key = tuple(sorted(cfg.items())) if cfg else ()
    if key not in _CACHE:
        _CACHE[key] = _build(key)
    return _CACHE[key]


# ------------------------------------------------------------------ wrapper
def kernel(_cfg=None, _want_results=False, **inputs):
    from concourse.bass_utils import run_bass_kernel_spmd

    inputs = {k: np.asarray(v, dtype=np.float32) for k, v in inputs.items()}
    nc = _get_program({})
    in_maps = [_host_prep(core, inputs) for core in range(8)]
    res = run_bass_kernel_spmd(nc, in_maps, list(range(8)))

    out = np.empty((B, N, DIM), np.float32)
    pb = inputs["proj_b"]
    for b in range(B):
        out[b] = (res.results[2 * b]["out"].astype(np.float32)
                  + res.results[2 * b + 1]["out"].astype(np.float32) + pb)
    if _want_results:
        return out, res
    return out


# revision 15
# speedup vs baseline: 1.4047x; 1.4047x over previous
"""Trainium2 Bass kernel for nn_AttentionWithVQ (B=4, N=2048, DIM=512, H=8,
depthwise-conv positional term, softmax attention, output projection).

Sharding: data-parallel over B (4 batches x 2 core-groups) and tensor-parallel
over heads (4 heads per core) -> 8 cores, fully independent per core except a
final partial-sum over the two head-groups of each batch, done on host at
gather time (the output projection contracts over heads).

Core algorithmic fusion: the score matrix
    S = 0.5*(scale * q @ k^T + scale * conv1(m) @ conv2(s)^T)
is ONE matmul over a concatenated 128-feature axis:
    S = Qp^T @ Kp,  Qp = [q*scale*0.5 ; conv1(m)*scale*0.5], Kp = [k ; conv2(s)]
which exactly fills the 128x128 PE array contraction dim.

Softmax denominators come for free by appending a ones-column to V
(attn@[V|1] yields the row-sums of exp(S) in the last output row); exp() is
numerically safe without max-subtraction for this problem's score magnitudes.

Schedule: the kernel is paced by the Scalar engine's 128 exp() instructions
(the hard floor at ~1.1us each).  Everything else is arranged around keeping
that stream dense:
  - minimal prologue: only the qkv chunks needed by head 0/1 stripe 0 run
    before the first exp; v-projection, the remaining qkv chunks, the t=1
    convs and the previous stripe's output projection are emitted as PE/DVE
    "fillers" inside the attention nk-loops.
  - loop order stripe-outer/head-inner so each stripe's projection + output
    DMA overlaps the next stripe's attention (no serialized tail).
  - per-(head,stripe) softmax normalization (reciprocal + DRAM-bounce
    partition broadcast) overlapped with the next head's attention.

Partition alignment: compute engines are lane-locked, so per-head feature
layouts alternate by head parity (even heads [qk;conv], odd heads [conv;qk])
making every PSUM->SBUF copy partition-aligned; the few genuinely shifting
copies (odd-head attention outputs, denominator broadcast) go through DMA.
"""

import os
import sys

sys.path.insert(0, "/opt/trn_rl_repo")

import numpy as np

# ---------------------------------------------------------------- constants
B, N, DIM, HEAD, VQE_K = 4, 2048, 512, 8, 3
Dh = DIM // HEAD            # 64
HPC = HEAD // 2             # heads per core (8 cores = 4 batch * 2 groups)
P = 128
NKB = N // P                # 16 key blocks
FB = 512                    # one fp32 PSUM bank
FBS = 1024                  # attention stripe chunk (2 banks)
NST = N // FBS              # 2 q-stripes
SCALE_Q = Dh ** -0.5 * 0.5  # folds the 0.5 score scale into the q/conv1 side

_DEFAULT_CFG = {}
_CACHE = {}


# ---------------------------------------------------------------- host prep
def _host_prep(core, inp):
    """Build the per-core input arrays (sharding + layout permutations)."""
    import ml_dtypes

    bf16 = ml_dtypes.bfloat16
    b, g = core // 2, core % 2
    f32 = np.float32
    x, m, s = inp["x"], inp["m"], inp["s"]
    qkv_w, qkv_b = inp["qkv_w"], inp["qkv_b"]
    proj_w = inp["proj_w"]
    p1w = inp["pe1_w"].reshape(HEAD, VQE_K)
    p2w = inp["pe2_w"].reshape(HEAD, VQE_K)
    pe1_b, pe2_b = inp["pe1_b"], inp["pe2_b"]

    d = {}
    d["xt"] = np.ascontiguousarray(x[b].T).astype(bf16)  # [512, 2048]

    # m/s transposed, tile t rows = [head(2t+1) feats ; head(2t) feats]
    mt = np.empty((256, N), f32)
    st = np.empty((256, N), f32)
    mcw = np.zeros((128, 8), f32)
    scw = np.zeros((128, 8), f32)
    for t in range(2):
        h_lo, h_hi = g * 4 + 2 * t + 1, g * 4 + 2 * t
        mt[t * 128:t * 128 + 64] = m[b][:, h_lo * 64:(h_lo + 1) * 64].T
        mt[t * 128 + 64:t * 128 + 128] = m[b][:, h_hi * 64:(h_hi + 1) * 64].T
        st[t * 128:t * 128 + 64] = s[b][:, h_lo * 64:(h_lo + 1) * 64].T
        st[t * 128 + 64:t * 128 + 128] = s[b][:, h_hi * 64:(h_hi + 1) * 64].T
        for p in range(128):
            h = g * 4 + 2 * t + (1 if p < 64 else 0)
            mcw[p, 4 * t:4 * t + 3] = p1w[h] * SCALE_Q
            scw[p, 4 * t:4 * t + 3] = p2w[h]
            mcw[p, 4 * t + 3] = pe1_b[h] * SCALE_Q
            scw[p, 4 * t + 3] = pe2_b[h]
    d["mt"], d["st"] = mt.astype(bf16), st.astype(bf16)
    d["mcw"], d["scw"] = mcw, scw

    # q/k projection weights: chunk ch=(t, q|k) = [even-head rows; odd-head rows]
    wqk_f = np.empty((512, DIM), f32)
    qkb = np.zeros((128, 4), f32)
    for t in range(2):
        for j in range(2):  # 0=q, 1=k
            ch = 2 * t + j
            h_e, h_o = g * 4 + 2 * t, g * 4 + 2 * t + 1
            base = j * DIM
            wqk_f[ch * 128:ch * 128 + 64] = qkv_w[base + h_e * 64:base + (h_e + 1) * 64]
            wqk_f[ch * 128 + 64:(ch + 1) * 128] = qkv_w[base + h_o * 64:base + (h_o + 1) * 64]
            qkb[0:64, ch] = qkv_b[base + h_e * 64:base + (h_e + 1) * 64]
            qkb[64:128, ch] = qkv_b[base + h_o * 64:base + (h_o + 1) * 64]
            if j == 0:
                wqk_f[ch * 128:(ch + 1) * 128] *= SCALE_Q
                qkb[:, ch] *= SCALE_Q
    d["wqk"] = np.ascontiguousarray(wqk_f.T).astype(bf16)  # [c=512, f=512]
    d["qkb"] = qkb

    d["wv"] = np.ascontiguousarray(
        qkv_w[2 * DIM + g * 256:2 * DIM + (g + 1) * 256].T).astype(bf16)  # [512, 256]
    # v bias replicated along partitions: column order matches wv columns
    vb = qkv_b[2 * DIM + g * 256:2 * DIM + (g + 1) * 256]
    d["vbrep"] = np.broadcast_to(vb, (128, 256)).astype(bf16).copy()

    # proj rows in aT partition order: aT tile t partition p -> head
    # 2t+(p>=64), d=p%64
    pjt = np.empty((256, DIM), f32)
    for t in range(2):
        for p in range(128):
            h_l = 2 * t + (1 if p >= 64 else 0)
            h = g * 4 + h_l
            pjt[t * 128 + p] = proj_w[:, h * 64 + (p % 64)]
    d["pjt"] = pjt.astype(bf16)
    return d


# ------------------------------------------------------------- device build
def _emit(tc, nc, io):
    from contextlib import ExitStack

    from concourse import mybir

    dt = mybir.dt
    f32 = dt.float32
    bf16 = dt.bfloat16
    AF = mybir.ActivationFunctionType
    ALU = mybir.AluOpType

    with ExitStack() as ctx:
        persist = ctx.enter_context(tc.tile_pool(name="persist", bufs=1))
        xtp = ctx.enter_context(tc.tile_pool(name="xtp", bufs=1))
        convp = ctx.enter_context(tc.tile_pool(name="convp", bufs=2))
        convyp = ctx.enter_context(tc.tile_pool(name="convyp", bufs=2))
        # PSUM: s_pool 2x2 banks, o_pool 1x2 banks, shp 2x1 bank = 8 banks
        s_pool = ctx.enter_context(
            tc.tile_pool(name="s_pool", bufs=2, space="PSUM"))
        o_pool = ctx.enter_context(
            tc.tile_pool(name="o_pool", bufs=1, space="PSUM"))
        shp = ctx.enter_context(tc.tile_pool(name="shp", bufs=2, space="PSUM"))
        esb = ctx.enter_context(tc.tile_pool(name="esb", bufs=8))
        stgp = ctx.enter_context(tc.tile_pool(name="stgp", bufs=2))
        denp = ctx.enter_context(tc.tile_pool(name="denp", bufs=2))
        bcp = ctx.enter_context(tc.tile_pool(name="bcp", bufs=2))
        obp = ctx.enter_context(tc.tile_pool(name="obp", bufs=3))

        # ---- persistent tiles
        wqk_sb = [persist.tile([128, 512], bf16, name=f"wqk{c}", tag=f"wqk{c}")
                  for c in range(4)]
        wv_sb = [persist.tile([128, 256], bf16, name=f"wv{c}", tag=f"wv{c}")
                 for c in range(4)]
        pjt_sb = [persist.tile([128, 512], bf16, name=f"pjt{f}", tag=f"pjt{f}")
                  for f in range(2)]
        mcw_sb = persist.tile([128, 8], f32, name="mcw", tag="mcw")
        scw_sb = persist.tile([128, 8], f32, name="scw", tag="scw")
        qkb_sb = persist.tile([128, 4], f32, name="qkb", tag="qkb")
        vbr_sb = persist.tile([128, 256], bf16, name="vbrep", tag="vbrep")
        QP = [persist.tile([128, N], bf16, name=f"QP{h}", tag=f"QP{h}")
              for h in range(HPC)]
        KP = [persist.tile([128, N], bf16, name=f"KP{h}", tag=f"KP{h}")
              for h in range(HPC)]
        # per-head V block is [v(64) | ones | zero-pad] = 66 columns (even
        # width keeps 4-byte operand alignment for bf16)
        v_sb = [persist.tile([128, HPC * 66], bf16, name=f"vsb{b_}",
                             tag=f"vsb{b_}") for b_ in range(NKB)]
        aT = [persist.tile([128, N], bf16, name=f"aT{t}", tag=f"aT{t}")
              for t in range(2)]
        xt_sb = [xtp.tile([128, N], bf16, name=f"xt{c}", tag=f"xt{c}")
                 for c in range(4)]

        # ---- input DMAs, priority order.  Single transfers run at ~23GB/s
        # on one DMA engine, so critical tiles are split into partition
        # halves that run on separate engines concurrently.
        def dma2(q, dst, src, parts=2):
            p = dst.shape[0] // parts
            for i in range(parts):
                q.dma_start(dst[i * p:(i + 1) * p, :], src[i * p:(i + 1) * p, :])

        cin0 = {}
        for src in ("st", "mt"):
            cin0[src] = convp.tile([128, N], bf16, name=f"ci_{src}0",
                                   tag="cin")
        dma2(nc.sync, cin0["st"][:], io["st"][0:128, :])
        dma2(nc.sync, xt_sb[0][:], io["xt"][0:128, :])
        dma2(nc.gpsimd, xt_sb[2][:], io["xt"][256:384, :])
        for c in range(4):
            dma2(nc.scalar, wqk_sb[c][:], io["wqk"][c * 128:(c + 1) * 128, :])
        dma2(nc.sync, cin0["mt"][:], io["mt"][0:128, :])
        dma2(nc.sync, xt_sb[1][:], io["xt"][128:256, :])
        dma2(nc.gpsimd, xt_sb[3][:], io["xt"][384:512, :])
        for c in range(4):
            nc.gpsimd.dma_start(wv_sb[c][:], io["wv"][c * 128:(c + 1) * 128, :])
        nc.scalar.dma_start(qkb_sb[:], io["qkb"][:, :])
        nc.scalar.dma_start(mcw_sb[:], io["mcw"][:, :])
        nc.scalar.dma_start(scw_sb[:], io["scw"][:, :])
        nc.gpsimd.dma_start(vbr_sb[:], io["vbrep"][:, :])
        nc.scalar.dma_start(pjt_sb[0][:], io["pjt"][0:128, :])
        nc.scalar.dma_start(pjt_sb[1][:], io["pjt"][128:256, :])

        # ---- helpers -----------------------------------------------------
        def conv_ops(src, wv_, dst, t, xin=None):
            """Depthwise 3-tap conv along N for tile t of m/s (DVE), writing
            the two parity halves into the QP/KP tiles."""
            if xin is None:
                xin = convp.tile([128, N], bf16, name=f"ci_{src}{t}",
                                 tag="cin")
                nc.sync.dma_start(xin[:], io[src][t * 128:(t + 1) * 128, :])
            y = convyp.tile([128, N], bf16, name=f"cy_{src}{t}", tag="cy")
            w0, w1, w2, cb = (wv_[:, 4 * t + k:4 * t + k + 1] for k in range(4))
            nc.vector.tensor_scalar(y[:], xin[:], w1, cb, ALU.mult, ALU.add)
            nc.vector.scalar_tensor_tensor(
                y[:, 1:], xin[:, :N - 1], w0, y[:, 1:], ALU.mult, ALU.add)
            nc.vector.scalar_tensor_tensor(
                y[:, :N - 1], xin[:, 1:], w2, y[:, :N - 1], ALU.mult, ALU.add)
            nc.vector.tensor_copy(dst[2 * t + 1][0:64, :], y[0:64, :])
            nc.vector.tensor_copy(dst[2 * t][64:128, :], y[64:128, :])

        def qkv_chunk(ch, qs, pool, tag, width):
            """q/k projection chunk ch over q-columns qs (width cols)."""
            for step in qkv_chunk_steps(ch, qs, pool, tag, width):
                step()

        def qkv_chunk_steps(ch, qs, pool, tag, width):
            """Same, but as a list of single-matmul emission steps so the
            chunk can be spread across attention iterations."""
            t, j = ch // 2, ch % 2
            dst = QP if j == 0 else KP
            nh = width // FB
            state = {}

            def mk(ih, c):
                def step():
                    if "ps" not in state:
                        state["ps"] = pool.tile([128, width], f32,
                                                name="psqk", tag=tag)
                    ps = state["ps"]
                    nc.tensor.matmul(
                        ps[:, ih * FB:(ih + 1) * FB],
                        wqk_sb[c][:, ch * 128:(ch + 1) * 128],
                        xt_sb[c][:, qs.start + ih * FB:qs.start + (ih + 1) * FB],
                        start=(c == 0), stop=(c == 3))
                    if ih == nh - 1 and c == 3:
                        nc.vector.tensor_scalar_add(
                            dst[2 * t][0:64, qs], ps[0:64, :],
                            qkb_sb[0:64, ch:ch + 1])
                        nc.vector.tensor_scalar_add(
                            dst[2 * t + 1][64:128, qs], ps[64:128, :],
                            qkb_sb[64:128, ch:ch + 1])
                return step

            return [mk(ih, c) for ih in range(nh) for c in range(4)]

        def v_block(blk):
            """v projection for key-block blk + bias + ones/pad columns."""
            bs = slice(blk * 128, (blk + 1) * 128)
            ps = shp.tile([128, 512], f32, name="psv", tag="sh")
            for c in range(4):
                nc.tensor.matmul(ps[:, 0:256], xt_sb[c][:, bs], wv_sb[c][:],
                                 start=(c == 0), stop=(c == 3))
            v3 = v_sb[blk].rearrange("p (h f) -> p h f", h=HPC)
            ps3 = ps.rearrange("p (h f) -> p h f", f=64)
            nc.vector.scalar_tensor_tensor(
                v3[:, :, 0:64], ps3[:, 0:HPC, :],
                1.0, vbr_sb.rearrange("p (h f) -> p h f", h=HPC),
                ALU.mult, ALU.add)
            nc.vector.memset(v3[:, :, 64:65], 1.0)
            nc.vector.memset(v3[:, :, 65:66], 0.0)

        def drain(h, q2, o_ps):
            """Release o_ps fast (one staging copy), then normalize by the
            softmax denominators into aT off the critical path."""
            t, odd = h // 2, h % 2
            cs = slice(q2 * FBS, (q2 + 1) * FBS)
            row = h * NST + q2
            # single PSUM read frees o_ps for the next head's accumulation
            stg = stgp.tile([65, FBS], bf16, name=f"stg{row}", tag="stg")
            nc.vector.tensor_copy(stg[:], o_ps[0:65, :])
            den = denp.tile([128, FBS], bf16, name=f"den{row}", tag="den")
            with nc.allow_low_precision(reason="softmax denom fits bf16"):
                nc.vector.reciprocal(den[64:65, :], stg[64:65, :])
            nc.sync.dma_start(io["drec"][row:row + 1, :], den[64:65, :])
            bc = bcp.tile([64, FBS], bf16, name=f"bc{row}", tag="bc")
            for i in range(2):
                nc.sync.dma_start(
                    bc[i * 32:(i + 1) * 32, :],
                    io["drec"][row:row + 1, :].broadcast_to([32, FBS]))
            if odd:
                nc.vector.tensor_mul(stg[0:64, :], stg[0:64, :], bc[:])
                for i in range(2):
                    nc.sync.dma_start(aT[t][64 + i * 32:64 + (i + 1) * 32, cs],
                                      stg[i * 32:(i + 1) * 32, :])
            else:
                nc.vector.tensor_mul(aT[t][0:64, cs], stg[0:64, :], bc[:])

        def proj_block_steps(blk):
            """Output projection for 128 q rows + bf16 store, as 2 steps."""
            bs = slice(blk * 128, (blk + 1) * 128)
            state = {}

            def s0():
                state["pj"] = shp.tile([128, FB], f32, name="pj", tag="sh")
                nc.tensor.matmul(state["pj"][:], aT[0][:, bs], pjt_sb[0][:],
                                 start=True, stop=False)

            def s1():
                pj = state["pj"]
                nc.tensor.matmul(pj[:], aT[1][:, bs], pjt_sb[1][:],
                                 start=False, stop=True)
                ob = obp.tile([128, FB], bf16, name="ob", tag="ob")
                nc.vector.tensor_copy(ob[:], pj[:])
                nc.gpsimd.dma_start(io["out"][bs.start:bs.start + 64, :],
                                    ob[0:64, :])
                nc.gpsimd.dma_start(io["out"][bs.start + 64:bs.stop, :],
                                    ob[64:128, :])

            return [s0, s1]

        def proj_block(blk):
            for s in proj_block_steps(blk):
                s()

        # ---- prologue: conv t=0 + the qkv chunks head 0/1 stripe 0 needs,
        # plus q(t1) stripe 0 to keep later filler slots light
        conv_ops("st", scw_sb, KP, 0, cin0["st"])
        conv_ops("mt", mcw_sb, QP, 0, cin0["mt"])
        qkv_chunk(1, slice(0, 1024), s_pool, "sps", 1024)     # k(t0) cols 0:1024
        qkv_chunk(1, slice(1024, 2048), s_pool, "sps", 1024)  # k(t0) cols 1024:
        qkv_chunk(0, slice(0, 1024), s_pool, "sps", 1024)     # q(t0) cols 0:1024
        v_block(0)
        v_block(1)
        qkv_chunk(2, slice(0, 1024), s_pool, "sps", 1024)     # q(t1) cols 0:1024

        # ---- attention: stripe-outer, head-inner, exp-paced.  Fillers are
        # single-matmul-sized emission steps, one consumed per nk iteration.
        def fillers_for(h, q2):
            fl = []
            if q2 == 0 and h == 0:
                for blk in range(2, NKB):
                    fl.append(lambda b_=blk: v_block(b_))
                # t=1 convs ride the DVE queue during head 0
                fl.insert(6, lambda: conv_ops("st", scw_sb, KP, 1))
                fl.insert(11, lambda: conv_ops("mt", mcw_sb, QP, 1))
            elif q2 == 0 and h == 1:
                for qb in range(4):   # k(t1), 16 single-matmul steps
                    fl += qkv_chunk_steps(3, slice(qb * 512, (qb + 1) * 512),
                                          shp, "sh", 512)
            elif q2 == 0 and h == 2:
                # q(t1) and q(t0) stripe-1 columns (needed next stripe)
                for qb in (2, 3):
                    fl += qkv_chunk_steps(2, slice(qb * 512, (qb + 1) * 512),
                                          shp, "sh", 512)
                for qb in (2, 3):
                    fl += qkv_chunk_steps(0, slice(qb * 512, (qb + 1) * 512),
                                          shp, "sh", 512)
            elif q2 == 1 and h in (1, 3):
                # previous stripe's projection; pad the first slots so the
                # PE never head-of-line blocks on the preceding drain's DMA
                if h == 1:
                    fl += [None] * 6
                for blk in range(2 if h == 1 else 4, 4 if h == 1 else 8):
                    fl += proj_block_steps(blk)
            elif q2 == 1 and h == 0:
                fl += proj_block_steps(0)
                fl += proj_block_steps(1)
            return fl

        for q2 in range(NST):
            for h in ((0, 1, 2, 3) if q2 == 0 else (1, 3, 0, 2)):
                vcols = slice(h * 66, (h + 1) * 66)
                cs0 = q2 * FBS
                fl = fillers_for(h, q2)
                o_ps = o_pool.tile([66, FBS], f32, name=f"o{h}_{q2}",
                                   tag="ops")
                for nk in range(NKB):
                    if fl:
                        f = fl.pop(0)
                        if f is not None:
                            f()
                    ks = slice(nk * 128, (nk + 1) * 128)
                    s_ps = s_pool.tile([128, FBS], f32, name="sps", tag="sps")
                    for ih in range(2):
                        nc.tensor.matmul(
                            s_ps[:, ih * FB:(ih + 1) * FB], KP[h][:, ks],
                            QP[h][:, cs0 + ih * FB:cs0 + (ih + 1) * FB],
                            start=True, stop=True)
                    e = esb.tile([128, FBS], bf16, name="e", tag="e")
                    nc.scalar.activation(e[:], s_ps[:], AF.Exp)
                    for ih in range(2):
                        nc.tensor.matmul(
                            o_ps[:, ih * FB:(ih + 1) * FB], v_sb[nk][:, vcols],
                            e[:, ih * FB:(ih + 1) * FB],
                            start=(nk == 0), stop=(nk == NKB - 1))
                for f in fl:
                    if f is not None:
                        f()
                drain(h, q2, o_ps)
            if q2 == NST - 1:  # last stripe: projection is the tail
                for blk in range(q2 * 8, q2 * 8 + 8):
                    proj_block(blk)


def _build(cfg_key):
    from concourse import bacc, mybir, tile

    dt = mybir.dt
    nc = bacc.Bacc("TRN2", target_bir_lowering=False, debug=False,
                   num_devices=8)
    shapes = {
        "xt": ([DIM, N], dt.bfloat16),
        "mt": ([256, N], dt.bfloat16), "st": ([256, N], dt.bfloat16),
        "wqk": ([DIM, 512], dt.bfloat16), "wv": ([DIM, 256], dt.bfloat16),
        "pjt": ([256, DIM], dt.bfloat16),
        "mcw": ([128, 8], dt.float32), "scw": ([128, 8], dt.float32),
        "qkb": ([128, 4], dt.float32), "vbrep": ([128, 256], dt.bfloat16),
    }
    io = {}
    for name, (shape, dtt) in shapes.items():
        io[name] = nc.dram_tensor(name, shape, dtt,
                                  kind="ExternalInput").ap()
    io["out"] = nc.dram_tensor("out", [N, DIM], dt.bfloat16,
                               kind="ExternalOutput").ap()
    # internal DRAM bounce for the denominator broadcast (DMA cannot
    # replicate from an SBUF source, but a DRAM source AP is linear and
    # supports a zero-step leading dim)
    io["drec"] = nc.dram_tensor("drec", [HPC * NST, FBS], dt.bfloat16).ap()
    with tile.TileContext(nc) as tc:
        _emit(tc, nc, io)
    nc.compile()
    return nc


def _get_program(cfg=None):
    key = tuple(sorted(cfg.items())) if cfg else ()
    if key not in _CACHE:
        _CACHE[key] = _build(key)
    return _CACHE[key]


# ------------------------------------------------------------------ wrapper
def kernel(_cfg=None, _want_results=False, **inputs):
    from concourse.bass_utils import run_bass_kernel_spmd

    inputs = {k: np.asarray(v, dtype=np.float32) for k, v in inputs.items()}
    nc = _get_program({})
    in_maps = [_host_prep(core, inputs) for core in range(8)]
    res = run_bass_kernel_spmd(nc, in_maps, list(range(8)))

    out = np.empty((B, N, DIM), np.float32)
    pb = inputs["proj_b"]
    for b in range(B):
        out[b] = (res.results[2 * b]["out"].astype(np.float32)
                  + res.results[2 * b + 1]["out"].astype(np.float32) + pb)
    if _want_results:
        return out, res
    return out


# revision 21
# speedup vs baseline: 1.5753x; 1.1214x over previous
"""Trainium2 Bass kernel for nn_AttentionWithVQ (B=4, N=2048, DIM=512, H=8,
depthwise-conv positional term, softmax attention, output projection).

Sharding: data-parallel over B (4 batches x 2 core-groups) and tensor-parallel
over heads (4 heads per core) -> 8 cores, fully independent per core except a
final partial-sum over the two head-groups of each batch, done on host at
gather time (the output projection contracts over heads).

Core algorithmic fusion: the score matrix
    S = 0.5*(scale * q @ k^T + scale * conv1(m) @ conv2(s)^T)
is ONE matmul over a concatenated 128-feature axis:
    S = Qp^T @ Kp,  Qp = [q*scale*0.5 ; conv1(m)*scale*0.5], Kp = [k ; conv2(s)]
which exactly fills the 128x128 PE array contraction dim.

Softmax denominators come for free by appending a ones-column to V
(attn@[V|1] yields the row-sums of exp(S) in the last output row); exp() is
numerically safe without max-subtraction for this problem's score magnitudes.

Schedule: the kernel is paced by the Scalar engine's 128 exp() instructions
(the hard floor at ~1.1us each).  Everything else is arranged around keeping
that stream dense:
  - minimal prologue: only the qkv chunks needed by head 0/1 stripe 0 run
    before the first exp; v-projection, the remaining qkv chunks, the t=1
    convs and the previous stripe's output projection are emitted as PE/DVE
    "fillers" inside the attention nk-loops.
  - loop order stripe-outer/head-inner so each stripe's projection + output
    DMA overlaps the next stripe's attention (no serialized tail).
  - per-(head,stripe) softmax normalization (reciprocal + DRAM-bounce
    partition broadcast) overlapped with the next head's attention.

Partition alignment: compute engines are lane-locked, so per-head feature
layouts alternate by head parity (even heads [qk;conv], odd heads [conv;qk])
making every PSUM->SBUF copy partition-aligned; the few genuinely shifting
copies (odd-head attention outputs, denominator broadcast) go through DMA.
"""

import os
import sys

sys.path.insert(0, "/opt/trn_rl_repo")

import numpy as np

# ---------------------------------------------------------------- constants
B, N, DIM, HEAD, VQE_K = 4, 2048, 512, 8, 3
Dh = DIM // HEAD            # 64
HPC = HEAD // 2             # heads per core (8 cores = 4 batch * 2 groups)
P = 128
NKB = N // P                # 16 key blocks
FB = 512                    # one fp32 PSUM bank
FBS = 1024                  # attention stripe chunk (2 banks)
NST = N // FBS              # 2 q-stripes
SCALE_Q = Dh ** -0.5 * 0.5  # folds the 0.5 score scale into the q/conv1 side

_DEFAULT_CFG = {}
_CACHE = {}


# ---------------------------------------------------------------- host prep
def _host_prep(core, inp):
    """Build the per-core input arrays (sharding + layout permutations)."""
    import ml_dtypes

    bf16 = ml_dtypes.bfloat16
    b, g = core // 2, core % 2
    f32 = np.float32
    x, m, s = inp["x"], inp["m"], inp["s"]
    qkv_w, qkv_b = inp["qkv_w"], inp["qkv_b"]
    proj_w = inp["proj_w"]
    p1w = inp["pe1_w"].reshape(HEAD, VQE_K)
    p2w = inp["pe2_w"].reshape(HEAD, VQE_K)
    pe1_b, pe2_b = inp["pe1_b"], inp["pe2_b"]

    d = {}
    d["xt"] = np.ascontiguousarray(x[b].T).astype(bf16)  # [512, 2048]

    # m/s transposed, tile t rows = [head(2t+1) feats ; head(2t) feats]
    mt = np.empty((256, N), f32)
    st = np.empty((256, N), f32)
    mcw = np.zeros((128, 8), f32)
    scw = np.zeros((128, 8), f32)
    for t in range(2):
        h_lo, h_hi = g * 4 + 2 * t + 1, g * 4 + 2 * t
        mt[t * 128:t * 128 + 64] = m[b][:, h_lo * 64:(h_lo + 1) * 64].T
        mt[t * 128 + 64:t * 128 + 128] = m[b][:, h_hi * 64:(h_hi + 1) * 64].T
        st[t * 128:t * 128 + 64] = s[b][:, h_lo * 64:(h_lo + 1) * 64].T
        st[t * 128 + 64:t * 128 + 128] = s[b][:, h_hi * 64:(h_hi + 1) * 64].T
        for p in range(128):
            h = g * 4 + 2 * t + (1 if p < 64 else 0)
            mcw[p, 4 * t:4 * t + 3] = p1w[h] * SCALE_Q
            scw[p, 4 * t:4 * t + 3] = p2w[h]
            mcw[p, 4 * t + 3] = pe1_b[h] * SCALE_Q
            scw[p, 4 * t + 3] = pe2_b[h]
    d["mt"], d["st"] = mt.astype(bf16), st.astype(bf16)
    d["mcw"], d["scw"] = mcw, scw

    # q/k projection weights: chunk ch=(t, q|k) = [even-head rows; odd-head rows]
    wqk_f = np.empty((512, DIM), f32)
    qkb = np.zeros((128, 4), f32)
    for t in range(2):
        for j in range(2):  # 0=q, 1=k
            ch = 2 * t + j
            h_e, h_o = g * 4 + 2 * t, g * 4 + 2 * t + 1
            base = j * DIM
            wqk_f[ch * 128:ch * 128 + 64] = qkv_w[base + h_e * 64:base + (h_e + 1) * 64]
            wqk_f[ch * 128 + 64:(ch + 1) * 128] = qkv_w[base + h_o * 64:base + (h_o + 1) * 64]
            qkb[0:64, ch] = qkv_b[base + h_e * 64:base + (h_e + 1) * 64]
            qkb[64:128, ch] = qkv_b[base + h_o * 64:base + (h_o + 1) * 64]
            if j == 0:
                wqk_f[ch * 128:(ch + 1) * 128] *= SCALE_Q
                qkb[:, ch] *= SCALE_Q
    d["wqk"] = np.ascontiguousarray(wqk_f.T).astype(bf16)  # [c=512, f=512]
    d["qkb"] = qkb

    d["wv"] = np.ascontiguousarray(
        qkv_w[2 * DIM + g * 256:2 * DIM + (g + 1) * 256].T).astype(bf16)  # [512, 256]
    # v bias replicated along partitions: column order matches wv columns
    vb = qkv_b[2 * DIM + g * 256:2 * DIM + (g + 1) * 256]
    d["vbrep"] = np.broadcast_to(vb, (128, 256)).astype(bf16).copy()

    # proj rows in aT partition order: aT tile t partition p -> head
    # 2t+(p>=64), d=p%64
    pjt = np.empty((256, DIM), f32)
    for t in range(2):
        for p in range(128):
            h_l = 2 * t + (1 if p >= 64 else 0)
            h = g * 4 + h_l
            pjt[t * 128 + p] = proj_w[:, h * 64 + (p % 64)]
    d["pjt"] = pjt.astype(bf16)
    return d


# ------------------------------------------------------------- device build
def _emit(tc, nc, io):
    from contextlib import ExitStack

    from concourse import mybir

    dt = mybir.dt
    f32 = dt.float32
    bf16 = dt.bfloat16
    AF = mybir.ActivationFunctionType
    ALU = mybir.AluOpType

    with ExitStack() as ctx:
        persist = ctx.enter_context(tc.tile_pool(name="persist", bufs=1))
        xtp = ctx.enter_context(tc.tile_pool(name="xtp", bufs=1))
        convp = ctx.enter_context(tc.tile_pool(name="convp", bufs=2))
        convyp = ctx.enter_context(tc.tile_pool(name="convyp", bufs=2))
        # PSUM: s_pool 2x2 banks, o_pool 1x2 banks, shp 2x1 bank = 8 banks
        s_pool = ctx.enter_context(
            tc.tile_pool(name="s_pool", bufs=2, space="PSUM"))
        o_pool = ctx.enter_context(
            tc.tile_pool(name="o_pool", bufs=1, space="PSUM"))
        shp = ctx.enter_context(tc.tile_pool(name="shp", bufs=2, space="PSUM"))
        esb = ctx.enter_context(tc.tile_pool(name="esb", bufs=8))
        stgp = ctx.enter_context(tc.tile_pool(name="stgp", bufs=2))
        denp = ctx.enter_context(tc.tile_pool(name="denp", bufs=2))
        bcp = ctx.enter_context(tc.tile_pool(name="bcp", bufs=2))
        obp = ctx.enter_context(tc.tile_pool(name="obp", bufs=3))

        # ---- persistent tiles
        wqk_sb = [persist.tile([128, 512], bf16, name=f"wqk{c}", tag=f"wqk{c}")
                  for c in range(4)]
        wv_sb = [persist.tile([128, 256], bf16, name=f"wv{c}", tag=f"wv{c}")
                 for c in range(4)]
        pjt_sb = [persist.tile([128, 512], bf16, name=f"pjt{f}", tag=f"pjt{f}")
                  for f in range(2)]
        mcw_sb = persist.tile([128, 8], f32, name="mcw", tag="mcw")
        scw_sb = persist.tile([128, 8], f32, name="scw", tag="scw")
        qkb_sb = persist.tile([128, 4], f32, name="qkb", tag="qkb")
        vbr_sb = persist.tile([128, 256], bf16, name="vbrep", tag="vbrep")
        QP = [persist.tile([128, N], bf16, name=f"QP{h}", tag=f"QP{h}")
              for h in range(HPC)]
        KP = [persist.tile([128, N], bf16, name=f"KP{h}", tag=f"KP{h}")
              for h in range(HPC)]
        # per-head V block is [v(64) | ones | zero-pad] = 66 columns (even
        # width keeps 4-byte operand alignment for bf16)
        v_sb = [persist.tile([128, HPC * 66], bf16, name=f"vsb{b_}",
                             tag=f"vsb{b_}") for b_ in range(NKB)]
        aT = [persist.tile([128, N], bf16, name=f"aT{t}", tag=f"aT{t}")
              for t in range(2)]
        xt_sb = [xtp.tile([128, N], bf16, name=f"xt{c}", tag=f"xt{c}")
                 for c in range(4)]

        # ---- input DMAs, priority order.  Single transfers run at ~23GB/s
        # on one DMA engine, so critical tiles are split into partition
        # halves that run on separate engines concurrently.
        def dma2(q, dst, src, parts=2):
            p = dst.shape[0] // parts
            for i in range(parts):
                q.dma_start(dst[i * p:(i + 1) * p, :], src[i * p:(i + 1) * p, :])

        cin0 = {}
        for src in ("st", "mt"):
            cin0[src] = convp.tile([128, N], bf16, name=f"ci_{src}0",
                                   tag="cin")
        dma2(nc.sync, xt_sb[0][:], io["xt"][0:128, :])
        dma2(nc.gpsimd, xt_sb[2][:], io["xt"][256:384, :])
        for c in range(4):
            dma2(nc.scalar, wqk_sb[c][:], io["wqk"][c * 128:(c + 1) * 128, :])
        dma2(nc.sync, xt_sb[1][:], io["xt"][128:256, :])
        dma2(nc.gpsimd, xt_sb[3][:], io["xt"][384:512, :])
        dma2(nc.sync, cin0["st"][:], io["st"][0:128, :])
        dma2(nc.sync, cin0["mt"][:], io["mt"][0:128, :])
        for c in range(4):
            nc.gpsimd.dma_start(wv_sb[c][:], io["wv"][c * 128:(c + 1) * 128, :])
        nc.scalar.dma_start(qkb_sb[:], io["qkb"][:, :])
        nc.scalar.dma_start(mcw_sb[:], io["mcw"][:, :])
        nc.scalar.dma_start(scw_sb[:], io["scw"][:, :])
        nc.gpsimd.dma_start(vbr_sb[:], io["vbrep"][:, :])
        nc.scalar.dma_start(pjt_sb[0][:], io["pjt"][0:128, :])
        nc.scalar.dma_start(pjt_sb[1][:], io["pjt"][128:256, :])

        # ---- helpers -----------------------------------------------------
        def conv_ops(src, wv_, dst, t, xin=None):
            """Depthwise 3-tap conv along N for tile t of m/s (DVE), writing
            the two parity halves into the QP/KP tiles."""
            if xin is None:
                xin = convp.tile([128, N], bf16, name=f"ci_{src}{t}",
                                 tag="cin")
                nc.sync.dma_start(xin[:], io[src][t * 128:(t + 1) * 128, :])
            y = convyp.tile([128, N], bf16, name=f"cy_{src}{t}", tag="cy")
            w0, w1, w2, cb = (wv_[:, 4 * t + k:4 * t + k + 1] for k in range(4))
            nc.vector.tensor_scalar(y[:], xin[:], w1, cb, ALU.mult, ALU.add)
            nc.vector.scalar_tensor_tensor(
                y[:, 1:], xin[:, :N - 1], w0, y[:, 1:], ALU.mult, ALU.add)
            nc.vector.scalar_tensor_tensor(
                y[:, :N - 1], xin[:, 1:], w2, y[:, :N - 1], ALU.mult, ALU.add)
            nc.vector.tensor_copy(dst[2 * t + 1][0:64, :], y[0:64, :])
            nc.vector.tensor_copy(dst[2 * t][64:128, :], y[64:128, :])

        def qkv_chunk(ch, qs, pool, tag, width):
            """q/k projection chunk ch over q-columns qs (width cols)."""
            for step in qkv_chunk_steps(ch, qs, pool, tag, width):
                step()

        def qkv_chunk_steps(ch, qs, pool, tag, width):
            """Same, but as a list of single-matmul emission steps so the
            chunk can be spread across attention iterations."""
            t, j = ch // 2, ch % 2
            dst = QP if j == 0 else KP
            nh = width // FB
            state = {}

            def mk(ih, c):
                def step():
                    if "ps" not in state:
                        state["ps"] = pool.tile([128, width], f32,
                                                name="psqk", tag=tag)
                    ps = state["ps"]
                    nc.tensor.matmul(
                        ps[:, ih * FB:(ih + 1) * FB],
                        wqk_sb[c][:, ch * 128:(ch + 1) * 128],
                        xt_sb[c][:, qs.start + ih * FB:qs.start + (ih + 1) * FB],
                        start=(c == 0), stop=(c == 3))
                    if ih == nh - 1 and c == 3:
                        nc.vector.tensor_scalar_add(
                            dst[2 * t][0:64, qs], ps[0:64, :],
                            qkb_sb[0:64, ch:ch + 1])
                        nc.vector.tensor_scalar_add(
                            dst[2 * t + 1][64:128, qs], ps[64:128, :],
                            qkb_sb[64:128, ch:ch + 1])
                return step

            return [mk(ih, c) for ih in range(nh) for c in range(4)]

        def v_block(blk):
            """v projection for key-block blk + bias + ones/pad columns."""
            bs = slice(blk * 128, (blk + 1) * 128)
            ps = shp.tile([128, 512], f32, name="psv", tag="sh")
            for c in range(4):
                nc.tensor.matmul(ps[:, 0:256], xt_sb[c][:, bs], wv_sb[c][:],
                                 start=(c == 0), stop=(c == 3))
            v3 = v_sb[blk].rearrange("p (h f) -> p h f", h=HPC)
            ps3 = ps.rearrange("p (h f) -> p h f", f=64)
            nc.vector.scalar_tensor_tensor(
                v3[:, :, 0:64], ps3[:, 0:HPC, :],
                1.0, vbr_sb.rearrange("p (h f) -> p h f", h=HPC),
                ALU.mult, ALU.add)
            nc.vector.memset(v3[:, :, 64:65], 1.0)
            nc.vector.memset(v3[:, :, 65:66], 0.0)

        def drain(h, q2, o_ps, mul_eng=None):
            """Release o_ps fast (one staging copy), then normalize by the
            softmax denominators into aT off the critical path.  The
            reciprocal runs 64-partitions-wide via a DMA reshape (a [1,1024]
            single-lane reciprocal costs 6.5us); the multiply defaults to
            the otherwise-idle GpSimd engine."""
            t, odd = h // 2, h % 2
            if mul_eng is None:
                mul_eng = nc.gpsimd
            cs = slice(q2 * FBS, (q2 + 1) * FBS)
            row = h * NST + q2
            # single PSUM read frees o_ps for the next head's accumulation
            stg = stgp.tile([65, FBS], bf16, name=f"stg{row}", tag="stg")
            nc.vector.tensor_copy(stg[:], o_ps[0:65, :])
            den = denp.tile([64, FBS // 64], bf16, name=f"den{row}", tag="den")
            nc.sync.dma_start(den[:], stg[64:65, :])
            with nc.allow_low_precision(reason="softmax denom fits bf16"):
                nc.vector.reciprocal(den[:], den[:])
            nc.sync.dma_start(io["drec"][row:row + 1, :], den[:])
            bc = bcp.tile([64, FBS], bf16, name=f"bc{row}", tag="bc")
            for i in range(2):
                nc.sync.dma_start(
                    bc[i * 32:(i + 1) * 32, :],
                    io["drec"][row:row + 1, :].broadcast_to([32, FBS]))
            if odd:
                mul_eng.tensor_mul(stg[0:64, :], stg[0:64, :], bc[:])
                for i in range(2):
                    nc.sync.dma_start(aT[t][64 + i * 32:64 + (i + 1) * 32, cs],
                                      stg[i * 32:(i + 1) * 32, :])
            else:
                mul_eng.tensor_mul(aT[t][0:64, cs], stg[0:64, :], bc[:])

        def proj_block_steps(blk):
            """Output projection for 128 q rows + bf16 store, as 2 steps."""
            bs = slice(blk * 128, (blk + 1) * 128)
            state = {}

            def s0():
                state["pj"] = shp.tile([128, FB], f32, name="pj", tag="sh")
                nc.tensor.matmul(state["pj"][:], aT[0][:, bs], pjt_sb[0][:],
                                 start=True, stop=False)

            def s1():
                pj = state["pj"]
                nc.tensor.matmul(pj[:], aT[1][:, bs], pjt_sb[1][:],
                                 start=False, stop=True)
                ob = obp.tile([128, FB], bf16, name="ob", tag="ob")
                nc.vector.tensor_copy(ob[:], pj[:])
                nc.gpsimd.dma_start(io["out"][bs.start:bs.start + 64, :],
                                    ob[0:64, :])
                nc.sync.dma_start(io["out"][bs.start + 64:bs.stop, :],
                                  ob[64:128, :])

            return [s0, s1]

        def proj_block(blk):
            for s in proj_block_steps(blk):
                s()

        # ---- prologue: conv t=0 + the qkv chunks head 0/1 stripe 0 needs,
        # plus q(t1) stripe 0 to keep later filler slots light
        conv_ops("st", scw_sb, KP, 0, cin0["st"])
        conv_ops("mt", mcw_sb, QP, 0, cin0["mt"])
        qkv_chunk(1, slice(0, 1024), s_pool, "sps", 1024)     # k(t0) cols 0:1024
        qkv_chunk(1, slice(1024, 2048), s_pool, "sps", 1024)  # k(t0) cols 1024:
        qkv_chunk(0, slice(0, 1024), s_pool, "sps", 1024)     # q(t0) cols 0:1024
        v_block(0)
        v_block(1)

        # ---- attention: stripe-outer, head-inner, exp-paced.  Fillers are
        # single-matmul-sized emission steps, one consumed per nk iteration.
        def fillers_for(h, q2):
            fl = []
            if q2 == 0 and h == 0:
                for blk in range(2, NKB):
                    fl.append(lambda b_=blk: v_block(b_))
                # t=1 convs ride the DVE queue during head 0
                fl.insert(6, lambda: conv_ops("st", scw_sb, KP, 1))
                fl.insert(11, lambda: conv_ops("mt", mcw_sb, QP, 1))
            elif q2 == 0 and h == 1:
                # k(t1) cols 0:1024 and q(t1) cols 0:1024 — all due by h2
                for ch, qb in ((3, 0), (3, 1), (2, 0), (2, 1)):
                    fl += qkv_chunk_steps(ch, slice(qb * 512, (qb + 1) * 512),
                                          shp, "sh", 512)
            elif q2 == 0 and h == 2:
                # k(t1) tail (due by h2 iters 8/12) + q(t0) stripe-1 columns
                for ch, qb in ((3, 2), (3, 3), (0, 2), (0, 3)):
                    fl += qkv_chunk_steps(ch, slice(qb * 512, (qb + 1) * 512),
                                          shp, "sh", 512)
            elif q2 == 0 and h == 3:
                # q(t1) stripe-1 columns (due by stripe-1 h3)
                for ch, qb in ((2, 2), (2, 3)):
                    fl += qkv_chunk_steps(ch, slice(qb * 512, (qb + 1) * 512),
                                          shp, "sh", 512)
            elif q2 == 1 and h in (1, 3):
                # previous stripe's projection; pad the first slots so the
                # PE never head-of-line blocks on the preceding drain's DMA
                if h == 1:
                    fl += [None] * 6
                for blk in range(2 if h == 1 else 4, 4 if h == 1 else 8):
                    fl += proj_block_steps(blk)
            elif q2 == 1 and h == 0:
                fl += proj_block_steps(0)
                fl += proj_block_steps(1)
            return fl

        for q2 in range(NST):
            for h in ((0, 1, 2, 3) if q2 == 0 else (1, 3, 0, 2)):
                vcols = slice(h * 66, (h + 1) * 66)
                cs0 = q2 * FBS
                fl = fillers_for(h, q2)
                o_ps = o_pool.tile([66, FBS], f32, name=f"o{h}_{q2}",
                                   tag="ops")
                for nk in range(NKB):
                    if fl:
                        f = fl.pop(0)
                        if f is not None:
                            f()
                    ks = slice(nk * 128, (nk + 1) * 128)
                    s_ps = s_pool.tile([128, FBS], f32, name="sps", tag="sps")
                    for ih in range(2):
                        nc.tensor.matmul(
                            s_ps[:, ih * FB:(ih + 1) * FB], KP[h][:, ks],
                            QP[h][:, cs0 + ih * FB:cs0 + (ih + 1) * FB],
                            start=True, stop=True)
                    e = esb.tile([128, FBS], bf16, name="e", tag="e")
                    nc.scalar.activation(e[:], s_ps[:], AF.Exp)
                    for ih in range(2):
                        nc.tensor.matmul(
                            o_ps[:, ih * FB:(ih + 1) * FB], v_sb[nk][:, vcols],
                            e[:, ih * FB:(ih + 1) * FB],
                            start=(nk == 0), stop=(nk == NKB - 1))
                for f in fl:
                    if f is not None:
                        f()
                last = q2 == NST - 1 and h == 2
                drain(h, q2, o_ps, mul_eng=nc.vector if last else None)
            if q2 == NST - 1:  # last stripe: projection is the tail
                for blk in range(q2 * 8, q2 * 8 + 8):
                    proj_block(blk)


def _build(cfg_key):
    from concourse import bacc, mybir, tile

    dt = mybir.dt
    nc = bacc.Bacc("TRN2", target_bir_lowering=False, debug=False,
                   num_devices=8)
    shapes = {
        "xt": ([DIM, N], dt.bfloat16),
        "mt": ([256, N], dt.bfloat16), "st": ([256, N], dt.bfloat16),
        "wqk": ([DIM, 512], dt.bfloat16), "wv": ([DIM, 256], dt.bfloat16),
        "pjt": ([256, DIM], dt.bfloat16),
        "mcw": ([128, 8], dt.float32), "scw": ([128, 8], dt.float32),
        "qkb": ([128, 4], dt.float32), "vbrep": ([128, 256], dt.bfloat16),
    }
    io = {}
    for name, (shape, dtt) in shapes.items():
        io[name] = nc.dram_tensor(name, shape, dtt,
                                  kind="ExternalInput").ap()
    io["out"] = nc.dram_tensor("out", [N, DIM], dt.bfloat16,
                               kind="ExternalOutput").ap()
    # internal DRAM bounce for the denominator broadcast (DMA cannot
    # replicate from an SBUF source, but a DRAM source AP is linear and
    # supports a zero-step leading dim)
    io["drec"] = nc.dram_tensor("drec", [HPC * NST, FBS], dt.bfloat16).ap()
    with tile.TileContext(nc) as tc:
        _emit(tc, nc, io)
    nc.compile()
    return nc


def _get_program(cfg=None):
    key = tuple(sorted(cfg.items())) if cfg else ()
    if key not in _CACHE:
        _CACHE[key] = _build(key)
    return _CACHE[key]


# ------------------------------------------------------------------ wrapper
def kernel(_cfg=None, _want_results=False, **inputs):
    from concourse.bass_utils import run_bass_kernel_spmd

    inputs = {k: np.asarray(v, dtype=np.float32) for k, v in inputs.items()}
    nc = _get_program({})
    in_maps = [_host_prep(core, inputs) for core in range(8)]
    res = run_bass_kernel_spmd(nc, in_maps, list(range(8)))

    out = np.empty((B, N, DIM), np.float32)
    pb = inputs["proj_b"]
    for b in range(B):
        out[b] = (res.results[2 * b]["out"].astype(np.float32)
                  + res.results[2 * b + 1]["out"].astype(np.float32) + pb)
    if _want_results:
        return out, res
    return out


# revision 22
# speedup vs baseline: 1.6164x; 1.0261x over previous
"""Trainium2 Bass kernel for nn_AttentionWithVQ (B=4, N=2048, DIM=512, H=8,
depthwise-conv positional term, softmax attention, output projection).

Sharding: data-parallel over B (4 batches x 2 core-groups) and tensor-parallel
over heads (4 heads per core) -> 8 cores, fully independent per core except a
final partial-sum over the two head-groups of each batch, done on host at
gather time (the output projection contracts over heads).

Core algorithmic fusion: the score matrix
    S = 0.5*(scale * q @ k^T + scale * conv1(m) @ conv2(s)^T)
is ONE matmul over a concatenated 128-feature axis:
    S = Qp^T @ Kp,  Qp = [q*scale*0.5 ; conv1(m)*scale*0.5], Kp = [k ; conv2(s)]
which exactly fills the 128x128 PE array contraction dim.

Softmax denominators come for free by appending a ones-column to V
(attn@[V|1] yields the row-sums of exp(S) in the last output row); exp() is
numerically safe without max-subtraction for this problem's score magnitudes.

Schedule: the kernel is paced by the Scalar engine's 128 exp() instructions
(the hard floor at ~1.1us each).  Everything else is arranged around keeping
that stream dense:
  - minimal prologue: only the qkv chunks needed by head 0/1 stripe 0 run
    before the first exp; v-projection, the remaining qkv chunks, the t=1
    convs and the previous stripe's output projection are emitted as PE/DVE
    "fillers" inside the attention nk-loops.
  - loop order stripe-outer/head-inner so each stripe's projection + output
    DMA overlaps the next stripe's attention (no serialized tail).
  - per-(head,stripe) softmax normalization (reciprocal + DRAM-bounce
    partition broadcast) overlapped with the next head's attention.

Partition alignment: compute engines are lane-locked, so per-head feature
layouts alternate by head parity (even heads [qk;conv], odd heads [conv;qk])
making every PSUM->SBUF copy partition-aligned; the few genuinely shifting
copies (odd-head attention outputs, denominator broadcast) go through DMA.
"""

import os
import sys

sys.path.insert(0, "/opt/trn_rl_repo")

import numpy as np

# ---------------------------------------------------------------- constants
B, N, DIM, HEAD, VQE_K = 4, 2048, 512, 8, 3
Dh = DIM // HEAD            # 64
HPC = HEAD // 2             # heads per core (8 cores = 4 batch * 2 groups)
P = 128
NKB = N // P                # 16 key blocks
FB = 512                    # one fp32 PSUM bank
FBS = 1024                  # attention stripe chunk (2 banks)
NST = N // FBS              # 2 q-stripes
SCALE_Q = Dh ** -0.5 * 0.5  # folds the 0.5 score scale into the q/conv1 side

_DEFAULT_CFG = {}
_CACHE = {}


# ---------------------------------------------------------------- host prep
def _host_prep(core, inp):
    """Build the per-core input arrays (sharding + layout permutations)."""
    import ml_dtypes

    bf16 = ml_dtypes.bfloat16
    b, g = core // 2, core % 2
    f32 = np.float32
    x, m, s = inp["x"], inp["m"], inp["s"]
    qkv_w, qkv_b = inp["qkv_w"], inp["qkv_b"]
    proj_w = inp["proj_w"]
    p1w = inp["pe1_w"].reshape(HEAD, VQE_K)
    p2w = inp["pe2_w"].reshape(HEAD, VQE_K)
    pe1_b, pe2_b = inp["pe1_b"], inp["pe2_b"]

    d = {}
    d["xt"] = np.ascontiguousarray(x[b].T).astype(bf16)  # [512, 2048]

    # m/s transposed, tile t rows = [head(2t+1) feats ; head(2t) feats]
    mt = np.empty((256, N), f32)
    st = np.empty((256, N), f32)
    mcw = np.zeros((128, 8), f32)
    scw = np.zeros((128, 8), f32)
    for t in range(2):
        h_lo, h_hi = g * 4 + 2 * t + 1, g * 4 + 2 * t
        mt[t * 128:t * 128 + 64] = m[b][:, h_lo * 64:(h_lo + 1) * 64].T
        mt[t * 128 + 64:t * 128 + 128] = m[b][:, h_hi * 64:(h_hi + 1) * 64].T
        st[t * 128:t * 128 + 64] = s[b][:, h_lo * 64:(h_lo + 1) * 64].T
        st[t * 128 + 64:t * 128 + 128] = s[b][:, h_hi * 64:(h_hi + 1) * 64].T
        for p in range(128):
            h = g * 4 + 2 * t + (1 if p < 64 else 0)
            mcw[p, 4 * t:4 * t + 3] = p1w[h] * SCALE_Q
            scw[p, 4 * t:4 * t + 3] = p2w[h]
            mcw[p, 4 * t + 3] = pe1_b[h] * SCALE_Q
            scw[p, 4 * t + 3] = pe2_b[h]
    d["mt"], d["st"] = mt.astype(bf16), st.astype(bf16)
    d["mcw"], d["scw"] = mcw, scw

    # q/k projection weights: chunk ch=(t, q|k) = [even-head rows; odd-head rows]
    wqk_f = np.empty((512, DIM), f32)
    qkb = np.zeros((128, 4), f32)
    for t in range(2):
        for j in range(2):  # 0=q, 1=k
            ch = 2 * t + j
            h_e, h_o = g * 4 + 2 * t, g * 4 + 2 * t + 1
            base = j * DIM
            wqk_f[ch * 128:ch * 128 + 64] = qkv_w[base + h_e * 64:base + (h_e + 1) * 64]
            wqk_f[ch * 128 + 64:(ch + 1) * 128] = qkv_w[base + h_o * 64:base + (h_o + 1) * 64]
            qkb[0:64, ch] = qkv_b[base + h_e * 64:base + (h_e + 1) * 64]
            qkb[64:128, ch] = qkv_b[base + h_o * 64:base + (h_o + 1) * 64]
            if j == 0:
                wqk_f[ch * 128:(ch + 1) * 128] *= SCALE_Q
                qkb[:, ch] *= SCALE_Q
    d["wqk"] = np.ascontiguousarray(wqk_f.T).astype(bf16)  # [c=512, f=512]
    d["qkb"] = qkb

    d["wv"] = np.ascontiguousarray(
        qkv_w[2 * DIM + g * 256:2 * DIM + (g + 1) * 256].T).astype(bf16)  # [512, 256]
    # v bias replicated along partitions: column order matches wv columns
    vb = qkv_b[2 * DIM + g * 256:2 * DIM + (g + 1) * 256]
    d["vbrep"] = np.broadcast_to(vb, (128, 256)).astype(bf16).copy()

    # proj rows in aT partition order: aT tile t partition p -> head
    # 2t+(p>=64), d=p%64
    pjt = np.empty((256, DIM), f32)
    for t in range(2):
        for p in range(128):
            h_l = 2 * t + (1 if p >= 64 else 0)
            h = g * 4 + h_l
            pjt[t * 128 + p] = proj_w[:, h * 64 + (p % 64)]
    d["pjt"] = pjt.astype(bf16)
    return d


# ------------------------------------------------------------- device build
def _emit(tc, nc, io):
    from contextlib import ExitStack

    from concourse import mybir

    dt = mybir.dt
    f32 = dt.float32
    bf16 = dt.bfloat16
    AF = mybir.ActivationFunctionType
    ALU = mybir.AluOpType

    with ExitStack() as ctx:
        persist = ctx.enter_context(tc.tile_pool(name="persist", bufs=1))
        xtp = ctx.enter_context(tc.tile_pool(name="xtp", bufs=1))
        convp = ctx.enter_context(tc.tile_pool(name="convp", bufs=2))
        convyp = ctx.enter_context(tc.tile_pool(name="convyp", bufs=2))
        # PSUM: s_pool 2x2 banks, o_pool 1x2 banks, shp 2x1 bank = 8 banks
        s_pool = ctx.enter_context(
            tc.tile_pool(name="s_pool", bufs=2, space="PSUM"))
        o_pool = ctx.enter_context(
            tc.tile_pool(name="o_pool", bufs=1, space="PSUM"))
        shp = ctx.enter_context(tc.tile_pool(name="shp", bufs=2, space="PSUM"))
        esb = ctx.enter_context(tc.tile_pool(name="esb", bufs=8))
        stgp = ctx.enter_context(tc.tile_pool(name="stgp", bufs=2))
        denp = ctx.enter_context(tc.tile_pool(name="denp", bufs=2))
        bcp = ctx.enter_context(tc.tile_pool(name="bcp", bufs=2))
        obp = ctx.enter_context(tc.tile_pool(name="obp", bufs=3))

        # ---- persistent tiles
        wqk_sb = [persist.tile([128, 512], bf16, name=f"wqk{c}", tag=f"wqk{c}")
                  for c in range(4)]
        wv_sb = [persist.tile([128, 256], bf16, name=f"wv{c}", tag=f"wv{c}")
                 for c in range(4)]
        pjt_sb = [persist.tile([128, 512], bf16, name=f"pjt{f}", tag=f"pjt{f}")
                  for f in range(2)]
        mcw_sb = persist.tile([128, 8], f32, name="mcw", tag="mcw")
        scw_sb = persist.tile([128, 8], f32, name="scw", tag="scw")
        qkb_sb = persist.tile([128, 4], f32, name="qkb", tag="qkb")
        vbr_sb = persist.tile([128, 256], bf16, name="vbrep", tag="vbrep")
        QP = [persist.tile([128, N], bf16, name=f"QP{h}", tag=f"QP{h}")
              for h in range(HPC)]
        KP = [persist.tile([128, N], bf16, name=f"KP{h}", tag=f"KP{h}")
              for h in range(HPC)]
        # per-head V block is [v(64) | ones | zero-pad] = 66 columns (even
        # width keeps 4-byte operand alignment for bf16)
        v_sb = [persist.tile([128, HPC * 66], bf16, name=f"vsb{b_}",
                             tag=f"vsb{b_}") for b_ in range(NKB)]
        aT = [persist.tile([128, N], bf16, name=f"aT{t}", tag=f"aT{t}")
              for t in range(2)]
        xt_sb = [xtp.tile([128, N], bf16, name=f"xt{c}", tag=f"xt{c}")
                 for c in range(4)]

        # ---- input DMAs, priority order.  Single transfers run at ~23GB/s
        # on one DMA engine, so critical tiles are split into partition
        # halves that run on separate engines concurrently.
        def dma2(q, dst, src, parts=2):
            p = dst.shape[0] // parts
            for i in range(parts):
                q.dma_start(dst[i * p:(i + 1) * p, :], src[i * p:(i + 1) * p, :])

        cin0 = {}
        for src in ("st", "mt"):
            cin0[src] = convp.tile([128, N], bf16, name=f"ci_{src}0",
                                   tag="cin")
        nc.scalar.dma_start(mcw_sb[:], io["mcw"][:, :])
        nc.scalar.dma_start(scw_sb[:], io["scw"][:, :])
        dma2(nc.sync, cin0["st"][:], io["st"][0:128, :])
        dma2(nc.gpsimd, cin0["mt"][:], io["mt"][0:128, :])
        dma2(nc.sync, xt_sb[0][:], io["xt"][0:128, :])
        dma2(nc.gpsimd, xt_sb[2][:], io["xt"][256:384, :])
        for c in range(4):
            dma2(nc.scalar, wqk_sb[c][:], io["wqk"][c * 128:(c + 1) * 128, :])
        dma2(nc.sync, xt_sb[1][:], io["xt"][128:256, :])
        dma2(nc.gpsimd, xt_sb[3][:], io["xt"][384:512, :])
        nc.scalar.dma_start(qkb_sb[:], io["qkb"][:, :])
        for c in range(4):
            nc.gpsimd.dma_start(wv_sb[c][:], io["wv"][c * 128:(c + 1) * 128, :])
        nc.gpsimd.dma_start(vbr_sb[:], io["vbrep"][:, :])
        nc.scalar.dma_start(pjt_sb[0][:], io["pjt"][0:128, :])
        nc.scalar.dma_start(pjt_sb[1][:], io["pjt"][128:256, :])

        # ---- helpers -----------------------------------------------------
        def conv_ops(src, wv_, dst, t, xin=None):
            """Depthwise 3-tap conv along N for tile t of m/s (DVE), writing
            the two parity halves into the QP/KP tiles."""
            if xin is None:
                xin = convp.tile([128, N], bf16, name=f"ci_{src}{t}",
                                 tag="cin")
                nc.sync.dma_start(xin[:], io[src][t * 128:(t + 1) * 128, :])
            y = convyp.tile([128, N], bf16, name=f"cy_{src}{t}", tag="cy")
            w0, w1, w2, cb = (wv_[:, 4 * t + k:4 * t + k + 1] for k in range(4))
            nc.vector.tensor_scalar(y[:], xin[:], w1, cb, ALU.mult, ALU.add)
            nc.vector.scalar_tensor_tensor(
                y[:, 1:], xin[:, :N - 1], w0, y[:, 1:], ALU.mult, ALU.add)
            nc.vector.scalar_tensor_tensor(
                y[:, :N - 1], xin[:, 1:], w2, y[:, :N - 1], ALU.mult, ALU.add)
            nc.vector.tensor_copy(dst[2 * t + 1][0:64, :], y[0:64, :])
            nc.vector.tensor_copy(dst[2 * t][64:128, :], y[64:128, :])

        def qkv_chunk(ch, qs, pool, tag, width):
            """q/k projection chunk ch over q-columns qs (width cols)."""
            for step in qkv_chunk_steps(ch, qs, pool, tag, width):
                step()

        def qkv_chunk_steps(ch, qs, pool, tag, width):
            """Same, but as a list of single-matmul emission steps so the
            chunk can be spread across attention iterations."""
            t, j = ch // 2, ch % 2
            dst = QP if j == 0 else KP
            nh = width // FB
            state = {}

            def mk(ih, c):
                def step():
                    if "ps" not in state:
                        state["ps"] = pool.tile([128, width], f32,
                                                name="psqk", tag=tag)
                    ps = state["ps"]
                    nc.tensor.matmul(
                        ps[:, ih * FB:(ih + 1) * FB],
                        wqk_sb[c][:, ch * 128:(ch + 1) * 128],
                        xt_sb[c][:, qs.start + ih * FB:qs.start + (ih + 1) * FB],
                        start=(c == 0), stop=(c == 3))
                    if ih == nh - 1 and c == 3:
                        nc.vector.tensor_scalar_add(
                            dst[2 * t][0:64, qs], ps[0:64, :],
                            qkb_sb[0:64, ch:ch + 1])
                        nc.vector.tensor_scalar_add(
                            dst[2 * t + 1][64:128, qs], ps[64:128, :],
                            qkb_sb[64:128, ch:ch + 1])
                return step

            return [mk(ih, c) for ih in range(nh) for c in range(4)]

        def v_block(blk):
            """v projection for key-block blk + bias + ones/pad columns."""
            bs = slice(blk * 128, (blk + 1) * 128)
            ps = shp.tile([128, 512], f32, name="psv", tag="sh")
            for c in range(4):
                nc.tensor.matmul(ps[:, 0:256], xt_sb[c][:, bs], wv_sb[c][:],
                                 start=(c == 0), stop=(c == 3))
            v3 = v_sb[blk].rearrange("p (h f) -> p h f", h=HPC)
            ps3 = ps.rearrange("p (h f) -> p h f", f=64)
            nc.vector.scalar_tensor_tensor(
                v3[:, :, 0:64], ps3[:, 0:HPC, :],
                1.0, vbr_sb.rearrange("p (h f) -> p h f", h=HPC),
                ALU.mult, ALU.add)
            nc.vector.memset(v3[:, :, 64:65], 1.0)
            nc.vector.memset(v3[:, :, 65:66], 0.0)

        def drain(h, q2, o_ps, mul_eng=None):
            """Release o_ps fast (one staging copy), then normalize by the
            softmax denominators into aT off the critical path.  The
            reciprocal runs 64-partitions-wide via a DMA reshape (a [1,1024]
            single-lane reciprocal costs 6.5us); the multiply defaults to
            the otherwise-idle GpSimd engine."""
            t, odd = h // 2, h % 2
            if mul_eng is None:
                mul_eng = nc.gpsimd
            cs = slice(q2 * FBS, (q2 + 1) * FBS)
            row = h * NST + q2
            # single PSUM read frees o_ps for the next head's accumulation
            stg = stgp.tile([65, FBS], bf16, name=f"stg{row}", tag="stg")
            nc.vector.tensor_copy(stg[:], o_ps[0:65, :])
            den = denp.tile([64, FBS // 64], bf16, name=f"den{row}", tag="den")
            nc.sync.dma_start(den[:], stg[64:65, :])
            with nc.allow_low_precision(reason="softmax denom fits bf16"):
                nc.vector.reciprocal(den[:], den[:])
            nc.sync.dma_start(io["drec"][row:row + 1, :], den[:])
            bc = bcp.tile([64, FBS], bf16, name=f"bc{row}", tag="bc")
            for i in range(2):
                nc.sync.dma_start(
                    bc[i * 32:(i + 1) * 32, :],
                    io["drec"][row:row + 1, :].broadcast_to([32, FBS]))
            if odd:
                mul_eng.tensor_mul(stg[0:64, :], stg[0:64, :], bc[:])
                for i in range(2):
                    nc.sync.dma_start(aT[t][64 + i * 32:64 + (i + 1) * 32, cs],
                                      stg[i * 32:(i + 1) * 32, :])
            else:
                mul_eng.tensor_mul(aT[t][0:64, cs], stg[0:64, :], bc[:])

        def proj_block_steps(blk):
            """Output projection for 128 q rows + bf16 store, as 2 steps."""
            bs = slice(blk * 128, (blk + 1) * 128)
            state = {}

            def s0():
                state["pj"] = shp.tile([128, FB], f32, name="pj", tag="sh")
                nc.tensor.matmul(state["pj"][:], aT[0][:, bs], pjt_sb[0][:],
                                 start=True, stop=False)

            def s1():
                pj = state["pj"]
                nc.tensor.matmul(pj[:], aT[1][:, bs], pjt_sb[1][:],
                                 start=False, stop=True)
                ob = obp.tile([128, FB], bf16, name="ob", tag="ob")
                nc.vector.tensor_copy(ob[:], pj[:])
                nc.gpsimd.dma_start(io["out"][bs.start:bs.start + 64, :],
                                    ob[0:64, :])
                nc.sync.dma_start(io["out"][bs.start + 64:bs.stop, :],
                                  ob[64:128, :])

            return [s0, s1]

        def proj_block(blk):
            for s in proj_block_steps(blk):
                s()

        # ---- prologue: conv t=0 + the qkv chunks head 0/1 stripe 0 needs,
        # plus q(t1) stripe 0 to keep later filler slots light
        conv_ops("st", scw_sb, KP, 0, cin0["st"])
        conv_ops("mt", mcw_sb, QP, 0, cin0["mt"])
        qkv_chunk(1, slice(0, 1024), s_pool, "sps", 1024)     # k(t0) cols 0:1024
        qkv_chunk(1, slice(1024, 2048), s_pool, "sps", 1024)  # k(t0) cols 1024:
        qkv_chunk(0, slice(0, 1024), s_pool, "sps", 1024)     # q(t0) cols 0:1024
        v_block(0)
        v_block(1)

        # ---- attention: stripe-outer, head-inner, exp-paced.  Fillers are
        # single-matmul-sized emission steps, one consumed per nk iteration.
        def fillers_for(h, q2):
            fl = []
            if q2 == 0 and h == 0:
                for blk in range(2, NKB):
                    fl.append(lambda b_=blk: v_block(b_))
                # t=1 convs ride the DVE queue during head 0
                fl.insert(6, lambda: conv_ops("st", scw_sb, KP, 1))
                fl.insert(11, lambda: conv_ops("mt", mcw_sb, QP, 1))
            elif q2 == 0 and h == 1:
                # k(t1) cols 0:1024 and q(t1) cols 0:1024 — all due by h2
                for ch, qb in ((3, 0), (3, 1), (2, 0), (2, 1)):
                    fl += qkv_chunk_steps(ch, slice(qb * 512, (qb + 1) * 512),
                                          shp, "sh", 512)
            elif q2 == 0 and h == 2:
                # k(t1) tail (due by h2 iters 8/12) + q(t0) stripe-1 columns
                for ch, qb in ((3, 2), (3, 3), (0, 2), (0, 3)):
                    fl += qkv_chunk_steps(ch, slice(qb * 512, (qb + 1) * 512),
                                          shp, "sh", 512)
            elif q2 == 0 and h == 3:
                # q(t1) stripe-1 columns (due by stripe-1 h3)
                for ch, qb in ((2, 2), (2, 3)):
                    fl += qkv_chunk_steps(ch, slice(qb * 512, (qb + 1) * 512),
                                          shp, "sh", 512)
            elif q2 == 1 and h in (1, 3):
                # previous stripe's projection; pad the first slots so the
                # PE never head-of-line blocks on the preceding drain's DMA
                if h == 1:
                    fl += [None] * 6
                for blk in range(2 if h == 1 else 4, 4 if h == 1 else 8):
                    fl += proj_block_steps(blk)
            elif q2 == 1 and h == 0:
                fl += proj_block_steps(0)
                fl += proj_block_steps(1)
            return fl

        for q2 in range(NST):
            for h in ((0, 1, 2, 3) if q2 == 0 else (1, 3, 0, 2)):
                vcols = slice(h * 66, (h + 1) * 66)
                cs0 = q2 * FBS
                fl = fillers_for(h, q2)
                o_ps = o_pool.tile([66, FBS], f32, name=f"o{h}_{q2}",
                                   tag="ops")
                for nk in range(NKB):
                    if fl:
                        f = fl.pop(0)
                        if f is not None:
                            f()
                    ks = slice(nk * 128, (nk + 1) * 128)
                    s_ps = s_pool.tile([128, FBS], f32, name="sps", tag="sps")
                    for ih in range(2):
                        nc.tensor.matmul(
                            s_ps[:, ih * FB:(ih + 1) * FB], KP[h][:, ks],
                            QP[h][:, cs0 + ih * FB:cs0 + (ih + 1) * FB],
                            start=True, stop=True)
                    e = esb.tile([128, FBS], bf16, name="e", tag="e")
                    nc.scalar.activation(e[:], s_ps[:], AF.Exp)
                    for ih in range(2):
                        nc.tensor.matmul(
                            o_ps[:, ih * FB:(ih + 1) * FB], v_sb[nk][:, vcols],
                            e[:, ih * FB:(ih + 1) * FB],
                            start=(nk == 0), stop=(nk == NKB - 1))
                for f in fl:
                    if f is not None:
                        f()
                last = q2 == NST - 1 and h == 2
                drain(h, q2, o_ps, mul_eng=nc.vector if last else None)
            if q2 == NST - 1:  # last stripe: projection is the tail
                for blk in range(q2 * 8, q2 * 8 + 8):
                    proj_block(blk)


def _build(cfg_key):
    from concourse import bacc, mybir, tile

    dt = mybir.dt
    nc = bacc.Bacc("TRN2", target_bir_lowering=False, debug=False,
                   num_devices=8)
    shapes = {
        "xt": ([DIM, N], dt.bfloat16),
        "mt": ([256, N], dt.bfloat16), "st": ([256, N], dt.bfloat16),
        "wqk": ([DIM, 512], dt.bfloat16), "wv": ([DIM, 256], dt.bfloat16),
        "pjt": ([256, DIM], dt.bfloat16),
        "mcw": ([128, 8], dt.float32), "scw": ([128, 8], dt.float32),
        "qkb": ([128, 4], dt.float32), "vbrep": ([128, 256], dt.bfloat16),
    }
    io = {}
    for name, (shape, dtt) in shapes.items():
        io[name] = nc.dram_tensor(name, shape, dtt,
                                  kind="ExternalInput").ap()
    io["out"] = nc.dram_tensor("out", [N, DIM], dt.bfloat16,
                               kind="ExternalOutput").ap()
    # internal DRAM bounce for the denominator broadcast (DMA cannot
    # replicate from an SBUF source, but a DRAM source AP is linear and
    # supports a zero-step leading dim)
    io["drec"] = nc.dram_tensor("drec", [HPC * NST, FBS], dt.bfloat16).ap()
    with tile.TileContext(nc) as tc:
        _emit(tc, nc, io)
    nc.compile()
    return nc


def _get_program(cfg=None):
    key = tuple(sorted(cfg.items())) if cfg else ()
    if key not in _CACHE:
        _CACHE[key] = _build(key)
    return _CACHE[key]


# ------------------------------------------------------------------ wrapper
def kernel(_cfg=None, _want_results=False, **inputs):
    from concourse.bass_utils import run_bass_kernel_spmd

    inputs = {k: np.asarray(v, dtype=np.float32) for k, v in inputs.items()}
    nc = _get_program({})
    in_maps = [_host_prep(core, inputs) for core in range(8)]
    res = run_bass_kernel_spmd(nc, in_maps, list(range(8)))

    out = np.empty((B, N, DIM), np.float32)
    pb = inputs["proj_b"]
    for b in range(B):
        out[b] = (res.results[2 * b]["out"].astype(np.float32)
                  + res.results[2 * b + 1]["out"].astype(np.float32) + pb)
    if _want_results:
        return out, res
    return out
